# revision 12
# baseline (speedup 1.0000x reference)
"""4D multilinear interpolation (8x8x8x8 lattice) on 8 Trainium2 cores.

For each row b: scale coordinates[b] (4 values in [0,1)) to the 7-cell
lattice, find the containing cell, gather the 16 corner values from
mesh_pred[b] (4096 values), and blend with multilinear weights.

Strategy (v4): Q7 SWDGE descriptor generation is the bottleneck
(~8ns/descriptor for int16-indexed dma_gather), so use exactly ONE
descriptor per row. The host relayouts each mesh row into 64
contiguous 256-float quads: quad beta = [block beta, beta+1, beta+8,
beta+9] (64-float blocks), an index-independent 4x duplication. One
1KB descriptor at beta = 8*ci0 + ci1 then fetches exactly the four
64-float windows holding the row's 16 corners: window (a,b) spans
lattice dims 2,3, which fold into a 64-wide dot with the rank-1
hat-weight vector U2 (x) U3 (one fused multiply per chunk + one
segmented tensor_reduce), then a tiny w0 (x) w1 combine. 8 dma_gather
instructions (512 rows each, int16 block indices) do all addressing;
indices/cell ids are host-precomputed metadata; all value math stays
on device in f32.
"""

import numpy as np

import concourse.bass as bass
import concourse.bacc as bacc
import concourse.mybir as mybir
from concourse import bass_utils, library_config
from concourse.tile import TileContext

P = 128            # partitions
N = 32             # row-columns per partition (P*N = 4096 rows/core)
BC = P * N         # rows per core
VOL = 4096         # 8^4 lattice values per row
NCORES = 8
GROUPS = 8         # dma_gather groups (int16 block-index range)
RPG = 512          # rows per group
NIDX = 512         # gather slots per group (1 quad/row)
ELEM = 256         # f32 elements per gathered quad (1KB)
STEP = 256         # f32 element stride between indexable quads
SLAB = 64 * ELEM   # relaid row size (16384 els)
NROWS_AP = 32768   # indexable quads per group window
GRP_ELS = RPG * SLAB
PAD = 1024
MESHN = BC * SLAB + PAD

F32 = mybir.dt.float32
I16 = mybir.dt.int16
OP = mybir.AluOpType


def _build():
    nc = bacc.Bacc("TRN2", target_bir_lowering=False, debug=False)
    mesh = nc.dram_tensor("mesh", [MESHN], F32, kind="ExternalInput")
    cw = nc.dram_tensor("cw", [P, 200], F32, kind="ExternalInput")
    idx = nc.dram_tensor("idx16", [P, GROUPS * 32], I16, kind="ExternalInput")
    out = nc.dram_tensor("out", [P, N], F32, kind="ExternalOutput")

    mesh_t = mesh[:].tensor

    with TileContext(nc) as tc:
        with tc.tile_pool(name="pool", bufs=1) as pool:
            nc.gpsimd.load_library(library_config.mlp)

            idx_t = pool.tile([P, GROUPS * 32], I16, tag="idx")
            nc.sync.dma_start(out=idx_t[:], in_=idx[:])
            cw_t = pool.tile([P, 200], F32, tag="cw")
            nc.sync.dma_start(out=cw_t[:], in_=cw[:])

            def view(tile_ap, off, dims):
                return bass.AP(tile_ap.tensor, tile_ap.offset + off, [tile_ap.ap[0]] + dims)

            # ---- coordinate math (all f32) ----
            # call: [p, d*32 + n] scaled coords, d-major
            call = pool.tile([P, 128], F32, tag="call")
            nc.vector.tensor_scalar_mul(call[:], cw_t[:, 0:128], 7.0)
            # WF: [p, s*64 + d*32 + n]; s=0 -> 1-frac, s=1 -> frac (d in {0,1})
            # cell ids ci01 for dims 0,1 arrive from the host (cols 128:192)
            WF = pool.tile([P, 128], F32, tag="WF")
            nc.vector.tensor_tensor(out=WF[:, 64:128], in0=call[:, 0:64],
                                    in1=cw_t[:, 128:192], op=OP.subtract)
            nc.vector.tensor_scalar(out=WF[:, 0:64], in0=WF[:, 64:128],
                                    scalar1=-1.0, scalar2=1.0,
                                    op0=OP.mult, op1=OP.add)

            # dims 2,3: hat weights U_d[q] = max(1 - |q - c_d|, 0), q in [0,8)
            # D23: [p, d*256 + n*8 + q] = iota_q - c_d  (d in {2,3})
            D23 = pool.tile([P, 512], F32, tag="D23")
            iota_v = view(cw_t[:], 192, [[0, 2], [0, 32], [1, 8]])
            c23_v = view(call[:], 64, [[32, 2], [1, 32], [0, 8]])
            nc.vector.tensor_tensor(out=D23[:], in0=iota_v, in1=c23_v, op=OP.subtract)
            # |.| and relu(1-|.|) on the otherwise-idle ACT engine
            bias1 = pool.tile([P, 1], F32, tag="bias1")
            nc.vector.memset(bias1[:], 1.0)
            N23 = pool.tile([P, 512], F32, tag="N23")
            nc.scalar.activation(out=N23[:], in_=D23[:],
                                 func=mybir.ActivationFunctionType.Abs)
            U23 = pool.tile([P, 512], F32, tag="U23")
            nc.scalar.activation(out=U23[:], in_=N23[:],
                                 func=mybir.ActivationFunctionType.Relu,
                                 bias=bias1[:, 0:1], scale=-1.0)
            # U64: [p, n*64 + Q*8 + R] = U2[n,Q] * U3[n,R]
            U64 = pool.tile([P, N * 64], F32, tag="U64")
            u2_v = view(U23[:], 0, [[8, 32], [1, 8], [0, 8]])
            u3_v = view(U23[:], 256, [[8, 32], [0, 8], [1, 8]])
            nc.vector.tensor_tensor(out=U64[:], in0=u2_v, in1=u3_v, op=OP.mult)

            # W01: [p, s*64 + b*32 + n] = w0[s,n] * w1[b,n]
            W01 = pool.tile([P, 128], F32, tag="W01")
            w0_v = view(WF[:], 0, [[64, 2], [0, 2], [1, 32]])
            w1_v = view(WF[:], 32, [[0, 2], [64, 2], [1, 32]])
            nc.vector.tensor_tensor(out=W01[:], in0=w0_v, in1=w1_v, op=OP.mult)

            # ---- gathers: 8 groups x 512 rows x 256-float quad ----
            # (last group split in two so the tail's transfer-completion
            # wait covers 256 rows instead of 512)
            Gbuf = pool.tile([P, GROUPS * 4 * ELEM], F32, tag="Gbuf")
            for g in range(GROUPS):
                halves = ((0, NIDX),) if g < GROUPS - 1 else ((0, 512), (256, 256))
                in_ap = bass.AP(mesh_t, g * GRP_ELS, [[STEP, NROWS_AP], [1, ELEM]])
                for off, cnt in halves:
                    if cnt == NIDX:
                        o = Gbuf[:, 4 * ELEM * g:4 * ELEM * (g + 1)]
                    else:
                        o = Gbuf[:, 4 * ELEM * g + 2 * ELEM * (off // 256):]
                        o = o[:, 0:2 * ELEM]
                    out3 = o.rearrange("p (k j) -> p k j", j=ELEM)
                    nc.gpsimd.dma_gather(
                        out_ap=out3,
                        in_ap=in_ap,
                        idxs_ap=idx_t[:, 32 * g + off // 16:32 * g + (off + cnt) // 16],
                        num_idxs=cnt,
                        num_idxs_reg=cnt,
                        elem_size=ELEM,
                        elem_step=STEP,
                    )

            # ---- blend: one fused M = G * U64 per group + segmented reduce ----
            E = pool.tile([P, 128], F32, tag="E")
            for c in range(8):
                Mc = pool.tile([P, 1024], F32, tag=f"M{c}")
                g_v = Gbuf[:, 1024 * c:1024 * (c + 1)]
                u_v = view(U64[:], c * 256, [[64, 4], [0, 4], [1, 64]])
                nc.vector.tensor_tensor(out=Mc[:], in0=g_v, in1=u_v, op=OP.mult)
                mc3 = Mc[:].rearrange("p (s j) -> p s j", s=16)
                nc.vector.tensor_reduce(out=E[:, 16 * c:16 * (c + 1)], in_=mc3,
                                        axis=mybir.AxisListType.X, op=OP.add)

            # T = E * W01: order ((g,m), (a,b))
            T = pool.tile([P, 128], F32, tag="T")
            w_v = view(W01[:], 0, [[1, 32], [32, 4]])
            nc.vector.scalar_tensor_tensor(out=T[:], in0=E[:], scalar=1.0,
                                           in1=w_v, op0=OP.mult, op1=OP.mult)
            T2 = pool.tile([P, 64], F32, tag="T2")
            nc.vector.tensor_tensor(out=T2[:], in0=T[:, 0::2], in1=T[:, 1::2],
                                    op=OP.add)
            acc = pool.tile([P, N], F32, tag="acc")
            nc.vector.tensor_tensor(out=acc[:], in0=T2[:, 0::2], in1=T2[:, 1::2],
                                    op=OP.add)
            nc.sync.dma_start(out=out[:], in_=acc[:])
    nc.compile()
    return nc


def _host_prep(coords_c, mesh_c):
    """Per-core input prep: coords+cells+iota, int16 quad indices, quad-mesh."""
    c7 = coords_c.astype(np.float32) * np.float32(7.0)
    ci = c7.astype(np.int32)            # trunc == floor (c >= 0); 0..6
    ci0, ci1 = ci[:, 0], ci[:, 1]

    i = np.arange(NIDX)
    p, m = i % P, i // P
    lr = 128 * m + p
    idx16 = np.zeros((16, GROUPS * 32), np.int16)
    for g in range(GROUPS):
        r = RPG * g + lr
        vals = 64 * lr + 8 * ci0[r] + ci1[r]
        idx16[i % 16, g * 32 + i // 16] = vals.astype(np.int16)
    idx16 = np.tile(idx16, (8, 1))

    cwA = coords_c.reshape(N, P, 4).transpose(1, 2, 0).reshape(P, 128)
    ciA = ci[:, 0:2].astype(np.float32).reshape(N, P, 2).transpose(1, 2, 0).reshape(P, 64)
    iot = np.broadcast_to(np.arange(8, dtype=np.float32), (P, 8))
    cw = np.ascontiguousarray(np.concatenate([cwA, ciA, iot], axis=1),
                              dtype=np.float32)

    # quad relayout: row -> 64 quads of [block b, b+1, b+8, b+9] (64-el blocks)
    B = mesh_c.reshape(BC, 64, 64)
    mesh_flat = np.zeros(MESHN, np.float32)
    Q = mesh_flat[:BC * SLAB].reshape(BC, 64, 4, 64)
    Q[:, :55, 0] = B[:, 0:55]
    Q[:, :55, 1] = B[:, 1:56]
    Q[:, :55, 2] = B[:, 8:63]
    Q[:, :55, 3] = B[:, 9:64]
    return {"mesh": mesh_flat, "cw": cw, "idx16": idx16}


_NC = None


def _get_nc():
    global _NC
    if _NC is None:
        _NC = _build()
    return _NC


def kernel(coordinates, mesh_pred, _trace=False, _tmpdir=None):
    coordinates = np.asarray(coordinates, dtype=np.float32)
    mesh_pred = np.asarray(mesh_pred, dtype=np.float32)
    assert coordinates.shape == (NCORES * BC, 4)
    assert mesh_pred.shape == (NCORES * BC, VOL)

    in_maps = []
    for cix in range(NCORES):
        sl = slice(cix * BC, (cix + 1) * BC)
        in_maps.append(_host_prep(coordinates[sl], mesh_pred[sl]))
    res = bass_utils.run_bass_kernel_spmd(
        _get_nc(),
        in_maps,
        core_ids=list(range(NCORES)),
        trace=_trace,
        tmpdir=_tmpdir,
    )
    outs = []
    for r in res.results:
        o = np.asarray(r["out"])              # [p, n]
        outs.append(o.transpose(1, 0).reshape(-1))  # row = n*P + p
    out = np.concatenate(outs)
    if _trace:
        return out, res
    return out


# revision 13
# speedup vs baseline: 1.0998x; 1.0998x over previous
"""4D multilinear interpolation (8x8x8x8 lattice) on 8 Trainium2 cores.

For each row b: scale coordinates[b] (4 values in [0,1)) to the 7-cell
lattice, find the containing cell, gather the 16 corner values from
mesh_pred[b] (4096 values), and blend with multilinear weights.

Strategy (v4): Q7 SWDGE descriptor generation is the bottleneck
(~8ns/descriptor for int16-indexed dma_gather), so use exactly ONE
descriptor per row. The host relayouts each mesh row into 64
contiguous 256-float quads: quad beta = [block beta, beta+1, beta+8,
beta+9] (64-float blocks), an index-independent 4x duplication. One
1KB descriptor at beta = 8*ci0 + ci1 then fetches exactly the four
64-float windows holding the row's 16 corners: window (a,b) spans
lattice dims 2,3, which fold into a 64-wide dot with the rank-1
hat-weight vector U2 (x) U3 (one fused multiply per chunk + one
segmented tensor_reduce), then a tiny w0 (x) w1 combine. 8 dma_gather
instructions (512 rows each, int16 block indices) do all addressing;
indices/cell ids are host-precomputed metadata; all value math stays
on device in f32.
"""

import numpy as np

import concourse.bass as bass
import concourse.bacc as bacc
import concourse.mybir as mybir
from concourse import bass_utils, library_config
from concourse.tile import TileContext

P = 128            # partitions
N = 32             # row-columns per partition (P*N = 4096 rows/core)
BC = P * N         # rows per core
VOL = 4096         # 8^4 lattice values per row
NCORES = 8
GROUPS = 8         # dma_gather groups (int16 block-index range)
RPG = 512          # rows per group
NIDX = 512         # gather slots per group (1 quad/row)
ELEM = 256         # f32 elements per gathered quad (1KB)
STEP = 256         # f32 element stride between indexable quads
SLAB = 64 * ELEM   # relaid row size (16384 els)
NROWS_AP = 32768   # indexable quads per group window
GRP_ELS = RPG * SLAB
PAD = 1024
MESHN = BC * SLAB + PAD

F32 = mybir.dt.float32
I16 = mybir.dt.int16
OP = mybir.AluOpType


def _build():
    nc = bacc.Bacc("TRN2", target_bir_lowering=False, debug=False)
    mesh = nc.dram_tensor("mesh", [MESHN], F32, kind="ExternalInput")
    cw = nc.dram_tensor("cw", [P, 200], F32, kind="ExternalInput")
    idx = nc.dram_tensor("idx16", [P, GROUPS * 32], I16, kind="ExternalInput")
    out = nc.dram_tensor("out", [P, N], F32, kind="ExternalOutput")

    mesh_t = mesh[:].tensor

    with TileContext(nc) as tc:
        with tc.tile_pool(name="pool", bufs=1) as pool:
            nc.gpsimd.load_library(library_config.mlp)

            idx_t = pool.tile([P, GROUPS * 32], I16, tag="idx")
            nc.sync.dma_start(out=idx_t[:], in_=idx[:])
            cw_t = pool.tile([P, 200], F32, tag="cw")
            nc.sync.dma_start(out=cw_t[:], in_=cw[:])

            def view(tile_ap, off, dims):
                return bass.AP(tile_ap.tensor, tile_ap.offset + off, [tile_ap.ap[0]] + dims)

            # ---- coordinate math (all f32) ----
            # call: [p, d*32 + n] scaled coords, d-major
            call = pool.tile([P, 128], F32, tag="call")
            nc.vector.tensor_scalar_mul(call[:], cw_t[:, 0:128], 7.0)
            # WF: [p, s*64 + d*32 + n]; s=0 -> 1-frac, s=1 -> frac (d in {0,1})
            # cell ids ci01 for dims 0,1 arrive from the host (cols 128:192)
            WF = pool.tile([P, 128], F32, tag="WF")
            nc.vector.tensor_tensor(out=WF[:, 64:128], in0=call[:, 0:64],
                                    in1=cw_t[:, 128:192], op=OP.subtract)
            nc.vector.tensor_scalar(out=WF[:, 0:64], in0=WF[:, 64:128],
                                    scalar1=-1.0, scalar2=1.0,
                                    op0=OP.mult, op1=OP.add)

            # dims 2,3: hat weights U_d[q] = max(1 - |q - c_d|, 0), q in [0,8)
            # D23: [p, d*256 + n*8 + q] = iota_q - c_d  (d in {2,3})
            D23 = pool.tile([P, 512], F32, tag="D23")
            iota_v = view(cw_t[:], 192, [[0, 2], [0, 32], [1, 8]])
            c23_v = view(call[:], 64, [[32, 2], [1, 32], [0, 8]])
            nc.vector.tensor_tensor(out=D23[:], in0=iota_v, in1=c23_v, op=OP.subtract)
            # |.| and relu(1-|.|) on the otherwise-idle ACT engine
            bias1 = pool.tile([P, 1], F32, tag="bias1")
            nc.vector.memset(bias1[:], 1.0)
            N23 = pool.tile([P, 512], F32, tag="N23")
            nc.scalar.activation(out=N23[:], in_=D23[:],
                                 func=mybir.ActivationFunctionType.Abs)
            U23 = pool.tile([P, 512], F32, tag="U23")
            nc.scalar.activation(out=U23[:], in_=N23[:],
                                 func=mybir.ActivationFunctionType.Relu,
                                 bias=bias1[:, 0:1], scale=-1.0)
            # U64: [p, n*64 + Q*8 + R] = U2[n,Q] * U3[n,R]
            U64 = pool.tile([P, N * 64], F32, tag="U64")
            u2_v = view(U23[:], 0, [[8, 32], [1, 8], [0, 8]])
            u3_v = view(U23[:], 256, [[8, 32], [0, 8], [1, 8]])
            nc.vector.tensor_tensor(out=U64[:], in0=u2_v, in1=u3_v, op=OP.mult)

            # W01: [p, s*64 + b*32 + n] = w0[s,n] * w1[b,n]
            W01 = pool.tile([P, 128], F32, tag="W01")
            w0_v = view(WF[:], 0, [[64, 2], [0, 2], [1, 32]])
            w1_v = view(WF[:], 32, [[0, 2], [64, 2], [1, 32]])
            nc.vector.tensor_tensor(out=W01[:], in0=w0_v, in1=w1_v, op=OP.mult)

            # ---- gathers: 8 groups x 512 rows x 256-float quad ----
            Gbuf = pool.tile([P, GROUPS * 4 * ELEM], F32, tag="Gbuf")
            for g in range(GROUPS):
                out3 = Gbuf[:, 4 * ELEM * g:4 * ELEM * (g + 1)].rearrange(
                    "p (k j) -> p k j", k=4)
                in_ap = bass.AP(mesh_t, g * GRP_ELS, [[STEP, NROWS_AP], [1, ELEM]])
                nc.gpsimd.dma_gather(
                    out_ap=out3,
                    in_ap=in_ap,
                    idxs_ap=idx_t[:, 32 * g:32 * (g + 1)],
                    num_idxs=NIDX,
                    num_idxs_reg=NIDX,
                    elem_size=ELEM,
                    elem_step=STEP,
                )

            # ---- blend: one fused M = G * U64 per group + segmented reduce ----
            E = pool.tile([P, 128], F32, tag="E")
            for c in range(8):
                Mc = pool.tile([P, 1024], F32, tag=f"M{c}")
                g_v = Gbuf[:, 1024 * c:1024 * (c + 1)]
                u_v = view(U64[:], c * 256, [[64, 4], [0, 4], [1, 64]])
                nc.vector.tensor_tensor(out=Mc[:], in0=g_v, in1=u_v, op=OP.mult)
                mc3 = Mc[:].rearrange("p (s j) -> p s j", s=16)
                nc.vector.tensor_reduce(out=E[:, 16 * c:16 * (c + 1)], in_=mc3,
                                        axis=mybir.AxisListType.X, op=OP.add)

            # T = E * W01: order ((g,m), (a,b))
            T = pool.tile([P, 128], F32, tag="T")
            w_v = view(W01[:], 0, [[1, 32], [32, 4]])
            nc.vector.scalar_tensor_tensor(out=T[:], in0=E[:], scalar=1.0,
                                           in1=w_v, op0=OP.mult, op1=OP.mult)
            T2 = pool.tile([P, 64], F32, tag="T2")
            nc.vector.tensor_tensor(out=T2[:], in0=T[:, 0::2], in1=T[:, 1::2],
                                    op=OP.add)
            acc = pool.tile([P, N], F32, tag="acc")
            nc.vector.tensor_tensor(out=acc[:], in0=T2[:, 0::2], in1=T2[:, 1::2],
                                    op=OP.add)
            nc.sync.dma_start(out=out[:], in_=acc[:])
    nc.compile()
    return nc


def _host_prep(coords_c, mesh_c):
    """Per-core input prep: coords+cells+iota, int16 quad indices, quad-mesh."""
    c7 = coords_c.astype(np.float32) * np.float32(7.0)
    ci = c7.astype(np.int32)            # trunc == floor (c >= 0); 0..6
    ci0, ci1 = ci[:, 0], ci[:, 1]

    i = np.arange(NIDX)
    p, m = i % P, i // P
    lr = 128 * m + p
    idx16 = np.zeros((16, GROUPS * 32), np.int16)
    for g in range(GROUPS):
        r = RPG * g + lr
        vals = 64 * lr + 8 * ci0[r] + ci1[r]
        idx16[i % 16, g * 32 + i // 16] = vals.astype(np.int16)
    idx16 = np.tile(idx16, (8, 1))

    cwA = coords_c.reshape(N, P, 4).transpose(1, 2, 0).reshape(P, 128)
    ciA = ci[:, 0:2].astype(np.float32).reshape(N, P, 2).transpose(1, 2, 0).reshape(P, 64)
    iot = np.broadcast_to(np.arange(8, dtype=np.float32), (P, 8))
    cw = np.ascontiguousarray(np.concatenate([cwA, ciA, iot], axis=1),
                              dtype=np.float32)

    # quad relayout: row -> 64 quads of [block b, b+1, b+8, b+9] (64-el blocks)
    B = mesh_c.reshape(BC, 64, 64)
    mesh_flat = np.zeros(MESHN, np.float32)
    Q = mesh_flat[:BC * SLAB].reshape(BC, 64, 4, 64)
    Q[:, :55, 0] = B[:, 0:55]
    Q[:, :55, 1] = B[:, 1:56]
    Q[:, :55, 2] = B[:, 8:63]
    Q[:, :55, 3] = B[:, 9:64]
    return {"mesh": mesh_flat, "cw": cw, "idx16": idx16}


_NC = None


def _get_nc():
    global _NC
    if _NC is None:
        _NC = _build()
    return _NC


def kernel(coordinates, mesh_pred, _trace=False, _tmpdir=None):
    coordinates = np.asarray(coordinates, dtype=np.float32)
    mesh_pred = np.asarray(mesh_pred, dtype=np.float32)
    assert coordinates.shape == (NCORES * BC, 4)
    assert mesh_pred.shape == (NCORES * BC, VOL)

    in_maps = []
    for cix in range(NCORES):
        sl = slice(cix * BC, (cix + 1) * BC)
        in_maps.append(_host_prep(coordinates[sl], mesh_pred[sl]))
    res = bass_utils.run_bass_kernel_spmd(
        _get_nc(),
        in_maps,
        core_ids=list(range(NCORES)),
        trace=_trace,
        tmpdir=_tmpdir,
    )
    outs = []
    for r in res.results:
        o = np.asarray(r["out"])              # [p, n]
        outs.append(o.transpose(1, 0).reshape(-1))  # row = n*P + p
    out = np.concatenate(outs)
    if _trace:
        return out, res
    return out
